# revision 9
# baseline (speedup 1.0000x reference)
"""VQ codebook argmin kernel for Trainium2 (8 NeuronCores, data-parallel over B).

Problem: x [32768, 512] f32, centroids [4096, 512] f32 ->
         argmin_k ||x_b - c_k||^2 = argmin_k (c_sq_k - 2 x.c_k)  -> [32768] int32

Sharding: x split along B into 8 shards of 4096 rows; centroids replicated.

Per-core algorithm (b-tile = 128 rows of x):
  prologue:
    - transpose centroids to cT [d=128 part, 4, 4096k] via PE transposes
    - csq_half_rep [128, 4096] = 0.5*sum_d c^2, replicated across partitions,
      computed with an all-ones stationary matmul over 0.5*cT^2
    - iota_rev [128, 4096] f32 = 4096 - k
  main loop over 32 b-tiles:
    - load x tile [128b, 512d], PE-transpose to xT [128d, 4, 128b]
    - for each of 8 k-chunks: 4 accumulating matmuls -> psum xc [128b, 512k]
    - fused DVE pass per chunk: dist = (xc - csq_half)*(-2) -> SBUF,
      accum chunk min
    - global min over 8 chunk mins
    - per chunk: (dist == gmin) * iota_rev, accum-sum -> cand (= 4096 - argmin)
    - idx = 4096 - max(cand); convert to int32
  epilogue: DMA [128, 32] int32 indices -> out[4096] (b = tile*128 + p)
"""
import sys

sys.path.insert(0, "/opt/trn_rl_repo")

import numpy as np

import concourse.bacc as bacc
import concourse.mybir as mybir
import concourse.tile as tile
from concourse.bass_utils import run_bass_kernel_spmd
from concourse.masks import make_identity

P = 128
D = 512
K = 4096
B = 32768
N_CORES = 8
B_SH = B // N_CORES          # 4096 rows per core
NBT = B_SH // P              # 32 b-tiles per core
DC = D // P                  # 4 contraction chunks
KC_SIZE = 512                # k-chunk (PSUM free dim)
NKC = K // KC_SIZE           # 8 k-chunks

MM_DT = mybir.dt.float32   # f32r probed lossy (tf32-like); f32 matmul is exact enough
MODE = "bf16x2"            # "f32" | "bf16x2"

F32 = mybir.dt.float32
BF16 = mybir.dt.bfloat16
AL = mybir.AluOpType


def build_bass(b_sh: int = B_SH, k: int = K, mm_dt=None):
    B_SH, K = b_sh, k          # shadow module constants for small test builds
    NBT = B_SH // P
    NKC = K // KC_SIZE
    MM_DT = mm_dt if mm_dt is not None else globals()["MM_DT"]

    nc = bacc.Bacc("TRN2", target_bir_lowering=False, debug=False)

    x_d = nc.dram_tensor("x_shard", [B_SH, D], F32, kind="ExternalInput")
    c_d = nc.dram_tensor("centroids", [K, D], F32, kind="ExternalInput")
    out_d = nc.dram_tensor("out_idx", [B_SH], mybir.dt.int32, kind="ExternalOutput")

    with tile.TileContext(nc) as tc:
        with (
            tc.tile_pool(name="persist", bufs=1) as persist,
            tc.tile_pool(name="cin", bufs=3) as cin,
            tc.tile_pool(name="xin", bufs=2) as xin,
            tc.tile_pool(name="dist", bufs=2) as distp,
            tc.tile_pool(name="small", bufs=3) as small,
            tc.tile_pool(name="scratch", bufs=2) as scratch,
            tc.tile_pool(name="mm_psum", bufs=4, space="PSUM") as mm_psum,
            tc.tile_pool(name="tr_psum", bufs=3, space="PSUM") as tr_psum,
        ):
            ident = persist.tile([P, P], F32)
            make_identity(nc, ident)

            iota_rev = persist.tile([P, K], F32)
            # iota_rev[p, k] = K - k  (channel_multiplier=0: same per partition)
            nc.gpsimd.iota(
                iota_rev[:],
                pattern=[[-1, K]],
                base=K,
                channel_multiplier=0,
                allow_small_or_imprecise_dtypes=True,
            )

            ones = persist.tile([P, P], F32)
            nc.vector.memset(ones[:], 1.0)

            # ---- transpose centroids: cT_all[dp, dc, k] = c[k, dc*128+dp]
            cT_all = persist.tile([P, DC, K], F32)
            for t in range(K // P):
                raw = cin.tile([P, D], F32, tag="raw_c")
                nc.sync.dma_start(raw[:], c_d.ap()[t * P:(t + 1) * P, :])
                for dc in range(DC):
                    pst = tr_psum.tile([P, P], F32, tag="tr")
                    nc.tensor.transpose(pst[:], raw[:, dc * P:(dc + 1) * P], ident[:])
                    nc.vector.tensor_copy(cT_all[:, dc, t * P:(t + 1) * P], pst[:])

            # ---- csq_rep[p, k] = sum_d c[k, d]^2 (same for all p)
            csq = persist.tile([P, K], F32)
            for j in range(NKC):
                ksl = slice(j * KC_SIZE, (j + 1) * KC_SIZE)
                sq = scratch.tile([P, DC, KC_SIZE], F32, tag="sq")
                nc.vector.tensor_tensor(
                    out=sq[:],
                    in0=cT_all[:, :, ksl],
                    in1=cT_all[:, :, ksl],
                    op=AL.mult,
                )
                ps = mm_psum.tile([P, KC_SIZE], F32, tag="mm")
                for dc in range(DC):
                    nc.tensor.matmul(
                        ps[:],
                        lhsT=ones[:].bitcast(MM_DT),
                        rhs=sq[:, dc, :].bitcast(MM_DT),
                        start=(dc == 0),
                        stop=(dc == DC - 1),
                    )
                nc.vector.tensor_copy(csq[:, ksl], ps[:])

            # ---- main loop over b-tiles
            idx_f32 = persist.tile([P, NBT], F32)
            for i in range(NBT):
                rawx = xin.tile([P, D], F32, tag="raw_x")
                nc.sync.dma_start(rawx[:], x_d.ap()[i * P:(i + 1) * P, :])
                xT = xin.tile([P, DC, P], F32, tag="xT")
                for dc in range(DC):
                    pst = tr_psum.tile([P, P], F32, tag="tr")
                    nc.tensor.transpose(pst[:], rawx[:, dc * P:(dc + 1) * P], ident[:])
                    nc.vector.tensor_copy(xT[:, dc, :], pst[:])

                dist = distp.tile([P, K], F32, tag="dist")
                cmin = small.tile([P, NKC], F32, tag="cmin")
                for j in range(NKC):
                    ksl = slice(j * KC_SIZE, (j + 1) * KC_SIZE)
                    ps = mm_psum.tile([P, KC_SIZE], F32, tag="mm")
                    for dc in range(DC):
                        nc.tensor.matmul(
                            ps[:],
                            lhsT=xT[:, dc, :].bitcast(MM_DT),
                            rhs=cT_all[:, dc, ksl].bitcast(MM_DT),
                            start=(dc == 0),
                            stop=(dc == DC - 1),
                        )
                    # dist = -2*xc + csq   (TTR is broken on this runtime;
                    # use STT + separate min-reduce)
                    nc.vector.scalar_tensor_tensor(
                        out=dist[:, ksl],
                        in0=ps[:],
                        scalar=-2.0,
                        in1=csq[:, ksl],
                        op0=AL.mult,
                        op1=AL.add,
                    )
                    nc.vector.tensor_reduce(
                        out=cmin[:, j:j + 1],
                        in_=dist[:, ksl],
                        axis=mybir.AxisListType.X,
                        op=AL.min,
                    )

                gmin = small.tile([P, 1], F32, tag="gmin")
                nc.vector.tensor_reduce(
                    out=gmin[:], in_=cmin[:], axis=mybir.AxisListType.X, op=AL.min
                )

                cand = small.tile([P, NKC], F32, tag="cand")
                for j in range(NKC):
                    ksl = slice(j * KC_SIZE, (j + 1) * KC_SIZE)
                    msk = scratch.tile([P, KC_SIZE], F32, tag="msk")
                    nc.vector.scalar_tensor_tensor(
                        out=msk[:],
                        in0=dist[:, ksl],
                        scalar=gmin[:],
                        in1=iota_rev[:, ksl],
                        op0=AL.is_equal,
                        op1=AL.mult,
                        accum_out=cand[:, j:j + 1],
                    )

                mrev = small.tile([P, 1], F32, tag="mrev")
                nc.vector.tensor_reduce(
                    out=mrev[:], in_=cand[:], axis=mybir.AxisListType.X, op=AL.max
                )
                # idx = K - mrev
                nc.vector.tensor_scalar(
                    idx_f32[:, i:i + 1], mrev[:], -1.0, float(K), AL.mult, AL.add
                )

            idx_i32 = persist.tile([P, NBT], mybir.dt.int32)
            nc.vector.tensor_copy(idx_i32[:], idx_f32[:])
            nc.sync.dma_start(
                out_d.ap().rearrange("(t p) -> p t", p=P), idx_i32[:]
            )

    nc.compile()
    return nc


def build_bass_bf16(b_sh: int = B_SH, k: int = K):
    """bf16 hi/lo split: xc = xh.ch + xh.cl + xl.ch (xl.cl term ~8.6e-5 abs,
    below the 3.2e-4 min argmin gap).  3 bf16 matmuls (1 cyc/col) vs fp32's
    4 cyc/col.  Phase-2 argmin scan runs on GpSimd to keep DVE under PE."""
    B_SH, K = b_sh, k
    NBT = B_SH // P
    NKC = K // KC_SIZE

    nc = bacc.Bacc("TRN2", target_bir_lowering=False, debug=False)

    x_d = nc.dram_tensor("x_shard", [B_SH, D], F32, kind="ExternalInput")
    c_d = nc.dram_tensor("centroids", [K, D], F32, kind="ExternalInput")
    out_d = nc.dram_tensor("out_idx", [B_SH], mybir.dt.int32, kind="ExternalOutput")

    with tile.TileContext(nc) as tc:
        with (
            tc.tile_pool(name="persist", bufs=1) as persist,
            tc.tile_pool(name="cin", bufs=3) as cin,
            tc.tile_pool(name="xin", bufs=2) as xin,
            tc.tile_pool(name="dist", bufs=2) as distp,
            tc.tile_pool(name="small", bufs=3) as small,
            tc.tile_pool(name="scratch", bufs=2) as scratch,
            tc.tile_pool(name="mm_psum", bufs=4, space="PSUM") as mm_psum,
            tc.tile_pool(name="tr_psum", bufs=4, space="PSUM") as tr_psum,
        ):
            identB = persist.tile([P, P], BF16)
            make_identity(nc, identB)

            iota_rev = persist.tile([P, K], F32)
            nc.gpsimd.iota(
                iota_rev[:],
                pattern=[[-1, K]],
                base=K,
                channel_multiplier=0,
                allow_small_or_imprecise_dtypes=True,
            )

            ones = persist.tile([P, P], F32)
            nc.vector.memset(ones[:], 1.0)

            # ---- split + transpose centroids
            chT = persist.tile([P, DC, K], BF16)
            clT = persist.tile([P, DC, K], BF16)
            for t in range(K // P):
                raw = cin.tile([P, D], F32, tag="raw_c")
                nc.sync.dma_start(raw[:], c_d.ap()[t * P:(t + 1) * P, :])
                ch = cin.tile([P, D], BF16, tag="ch")
                nc.vector.tensor_copy(ch[:], raw[:])
                cl = cin.tile([P, D], BF16, tag="cl")
                nc.vector.tensor_tensor(out=cl[:], in0=raw[:], in1=ch[:], op=AL.subtract)
                for dc in range(DC):
                    dsl = slice(dc * P, (dc + 1) * P)
                    tsl = slice(t * P, (t + 1) * P)
                    psh = tr_psum.tile([P, P], BF16, tag="tr")
                    nc.tensor.transpose(psh[:], ch[:, dsl], identB[:])
                    nc.vector.tensor_copy(chT[:, dc, tsl], psh[:])
                    psl = tr_psum.tile([P, P], BF16, tag="tr")
                    nc.tensor.transpose(psl[:], cl[:, dsl], identB[:])
                    nc.vector.tensor_copy(clT[:, dc, tsl], psl[:])

            # ---- csq_rep[p, k] = sum_d c^2 via ones-matmul over (chT+clT)^2
            csq = persist.tile([P, K], F32)
            for j in range(NKC):
                ksl = slice(j * KC_SIZE, (j + 1) * KC_SIZE)
                tmp = scratch.tile([P, DC, KC_SIZE], F32, tag="tmp")
                nc.vector.tensor_tensor(
                    out=tmp[:], in0=chT[:, :, ksl], in1=clT[:, :, ksl], op=AL.add
                )
                sq = scratch.tile([P, DC, KC_SIZE], F32, tag="sq")
                nc.scalar.activation(
                    sq[:], tmp[:], mybir.ActivationFunctionType.Square
                )
                ps = mm_psum.tile([P, KC_SIZE], F32, tag="mm")
                for dc in range(DC):
                    nc.tensor.matmul(
                        ps[:],
                        lhsT=ones[:],
                        rhs=sq[:, dc, :],
                        start=(dc == 0),
                        stop=(dc == DC - 1),
                    )
                nc.vector.tensor_copy(csq[:, ksl], ps[:])

            # ---- main loop
            idx_f32 = persist.tile([P, NBT], F32)
            for i in range(NBT):
                rawx = xin.tile([P, D], F32, tag="raw_x")
                nc.sync.dma_start(rawx[:], x_d.ap()[i * P:(i + 1) * P, :])
                xh = xin.tile([P, D], BF16, tag="xh")
                nc.vector.tensor_copy(xh[:], rawx[:])
                xl = xin.tile([P, D], BF16, tag="xl")
                nc.vector.tensor_tensor(out=xl[:], in0=rawx[:], in1=xh[:], op=AL.subtract)
                xhT = xin.tile([P, DC, P], BF16, tag="xhT")
                xlT = xin.tile([P, DC, P], BF16, tag="xlT")
                for dc in range(DC):
                    dsl = slice(dc * P, (dc + 1) * P)
                    psh = tr_psum.tile([P, P], BF16, tag="tr")
                    nc.tensor.transpose(psh[:], xh[:, dsl], identB[:])
                    nc.vector.tensor_copy(xhT[:, dc, :], psh[:])
                    psl = tr_psum.tile([P, P], BF16, tag="tr")
                    nc.tensor.transpose(psl[:], xl[:, dsl], identB[:])
                    nc.vector.tensor_copy(xlT[:, dc, :], psl[:])

                dist = distp.tile([P, K], F32, tag="dist")
                cmin = small.tile([P, NKC], F32, tag="cmin")
                for j in range(NKC):
                    ksl = slice(j * KC_SIZE, (j + 1) * KC_SIZE)
                    ps = mm_psum.tile([P, KC_SIZE], F32, tag="mm")
                    steps = [(xhT, chT), (xhT, clT), (xlT, chT)]
                    for s, (lt, rt) in enumerate(steps):
                        for dc in range(DC):
                            nc.tensor.matmul(
                                ps[:],
                                lhsT=lt[:, dc, :],
                                rhs=rt[:, dc, ksl],
                                start=(s == 0 and dc == 0),
                                stop=(s == len(steps) - 1 and dc == DC - 1),
                            )
                    nc.vector.scalar_tensor_tensor(
                        out=dist[:, ksl],
                        in0=ps[:],
                        scalar=-2.0,
                        in1=csq[:, ksl],
                        op0=AL.mult,
                        op1=AL.add,
                    )
                    nc.vector.tensor_reduce(
                        out=cmin[:, j:j + 1],
                        in_=dist[:, ksl],
                        axis=mybir.AxisListType.X,
                        op=AL.min,
                    )

                gmin = small.tile([P, 1], F32, tag="gmin")
                nc.vector.tensor_reduce(
                    out=gmin[:], in_=cmin[:], axis=mybir.AxisListType.X, op=AL.min
                )

                cand = small.tile([P, NKC], F32, tag="cand")
                for j in range(NKC):
                    ksl = slice(j * KC_SIZE, (j + 1) * KC_SIZE)
                    msk = scratch.tile([P, KC_SIZE], F32, tag="msk")
                    # walrus rejects TensorScalarPtr on Pool; DVE it is
                    nc.vector.scalar_tensor_tensor(
                        out=msk[:],
                        in0=dist[:, ksl],
                        scalar=gmin[:],
                        in1=iota_rev[:, ksl],
                        op0=AL.is_equal,
                        op1=AL.mult,
                        accum_out=cand[:, j:j + 1],
                    )

                mrev = small.tile([P, 1], F32, tag="mrev")
                nc.vector.tensor_reduce(
                    out=mrev[:], in_=cand[:], axis=mybir.AxisListType.X, op=AL.max
                )
                nc.vector.tensor_scalar(
                    idx_f32[:, i:i + 1], mrev[:], -1.0, float(K), AL.mult, AL.add
                )

            idx_i32 = persist.tile([P, NBT], mybir.dt.int32)
            nc.vector.tensor_copy(idx_i32[:], idx_f32[:])
            nc.sync.dma_start(
                out_d.ap().rearrange("(t p) -> p t", p=P), idx_i32[:]
            )

    nc.compile()
    return nc


_NC = None


def kernel(x: np.ndarray, centroids: np.ndarray) -> np.ndarray:
    global _NC
    if _NC is None:
        _NC = build_bass_bf16() if MODE == "bf16x2" else build_bass()
    x = np.ascontiguousarray(x, dtype=np.float32)
    centroids = np.ascontiguousarray(centroids, dtype=np.float32)
    in_maps = [
        {"x_shard": x[c * B_SH:(c + 1) * B_SH], "centroids": centroids}
        for c in range(N_CORES)
    ]
    res = run_bass_kernel_spmd(_NC, in_maps, core_ids=list(range(N_CORES)))
    return np.concatenate([res.results[c]["out_idx"] for c in range(N_CORES)])
